# revision 10
# baseline (speedup 1.0000x reference)
"""Bahdanau-style attention kernel for Trainium2, SPMD over 8 NeuronCores.

Problem (all fp32):
  hidden [B=32, H=1024], encoder_outputs [T=2048, B, H],
  W [H, 2H] (W1 | W2), b [H] (zeros), v [H]
  e    = tanh(hidden @ W1^T + enc @ W2^T + b)        [B, T, K=H]
  att  = e @ v                                       [B, T]
  out  = softmax(att, axis=T)[:, None, :]            [B, 1, T]

Sharding: data-parallel over B (4 batches per core), W/b/v replicated.

Per-core device algorithm (k on PSUM partitions, t on free dim):
  for b, t_tile(512), k_chunk(128):
      psum_e[k,t] = sum_{h_chunk} W2T[h,k].T @ encT[b][h,t]  (fp32r matmuls)
      e = tanh(psum_e + (s1[b]+bias)[k])                     (ACT, per-part bias)
      att_psum[1,t] += v[k_chunk].T @ e                      (fp32r matmul)
  softmax over t on [4, 2048] tile, DMA out.

s1 = hidden @ W1^T (+b) is 0.05% of the FLOPs and is precomputed on host.
"""

import numpy as np

B, T, H = 32, 2048, 1024
K = H
NCORES = 8
BC = B // NCORES  # batches per core
P = 128
HO = H // P       # 8 h-chunks
KO = K // P       # 8 k-chunks
TT = 512          # t tile (one PSUM bank of fp32)
NT = T // TT      # 4 t tiles


def build_program():
    from contextlib import ExitStack

    import concourse.tile as tile
    from concourse import bacc, mybir

    f32 = mybir.dt.float32
    f32r = mybir.dt.float32r
    AF = mybir.ActivationFunctionType

    nc = bacc.Bacc("TRN2", target_bir_lowering=False, debug=False)

    encT_d = nc.dram_tensor("encT", [BC, H, T], f32, kind="ExternalInput").ap()
    w2t_d = nc.dram_tensor("w2t", [H, K], f32, kind="ExternalInput").ap()
    s1b_d = nc.dram_tensor("s1b", [BC, K], f32, kind="ExternalInput").ap()
    v_d = nc.dram_tensor("v", [K], f32, kind="ExternalInput").ap()
    out_d = nc.dram_tensor("out", [BC, T], f32, kind="ExternalOutput").ap()

    with tile.TileContext(nc) as tc, ExitStack() as ctx:
        const = ctx.enter_context(tc.tile_pool(name="const", bufs=1))
        enc_pool = ctx.enter_context(tc.tile_pool(name="enc", bufs=3))
        e_pool = ctx.enter_context(tc.tile_pool(name="e", bufs=3))
        psum_pool = ctx.enter_context(tc.tile_pool(name="psum", bufs=2, space="PSUM"))
        att_psum_pool = ctx.enter_context(
            tc.tile_pool(name="attpsum", bufs=2, space="PSUM")
        )
        stat_pool = ctx.enter_context(tc.tile_pool(name="stat", bufs=1))

        # Resident weights: W2T [H, K] -> [hp=128, ho, k] (fp32r for the PE)
        w2t_sb = const.tile([P, HO, K], f32r)
        nc.sync.dma_start(
            w2t_sb[:], w2t_d.rearrange("(ho hp) k -> hp ho k", hp=P).bitcast(f32r)
        )
        # v [K] -> [kp=128, ko]
        v_sb = const.tile([P, KO], f32r)
        nc.sync.dma_start(v_sb[:], v_d.rearrange("(ko kp) -> kp ko", kp=P).bitcast(f32r))
        # s1 + bias [BC, K] -> [kp, b*KO+ko]
        s1b_sb = const.tile([P, BC * KO], f32)
        nc.sync.dma_start(
            s1b_sb[:], s1b_d.rearrange("b (ko kp) -> kp (b ko)", kp=P)
        )

        # attention energies gathered on partition 0 as [1, BC*T]
        att_row = const.tile([1, BC * T], f32)
        # then DMA-reshaped to [BC partitions, T] for the softmax
        att_sb = const.tile([BC, T], f32)

        for b in range(BC):
            for tt in range(NT):
                enc_sb = enc_pool.tile([P, HO, TT], f32r)
                nc.sync.dma_start(
                    enc_sb[:],
                    encT_d[b][:, tt * TT : (tt + 1) * TT]
                    .rearrange("(ho hp) t -> hp ho t", hp=P)
                    .bitcast(f32r),
                )
                att_psum = att_psum_pool.tile([1, TT], f32)
                for ko in range(KO):
                    psum_e = psum_pool.tile([P, TT], f32)
                    for ho in range(HO):
                        nc.tensor.matmul(
                            psum_e[:],
                            w2t_sb[:, ho, ko * P : (ko + 1) * P],
                            enc_sb[:, ho, :],
                            start=(ho == 0),
                            stop=(ho == HO - 1),
                        )
                    e_sb = e_pool.tile([P, TT], f32r)
                    nc.scalar.activation(
                        e_sb[:],
                        psum_e[:],
                        AF.Tanh,
                        bias=s1b_sb[:, b * KO + ko : b * KO + ko + 1],
                    )
                    nc.tensor.matmul(
                        att_psum[:],
                        v_sb[:, ko : ko + 1],
                        e_sb[:],
                        start=(ko == 0),
                        stop=(ko == KO - 1),
                    )
                # gather energies onto partition 0
                nc.vector.tensor_copy(
                    att_row[0:1, b * T + tt * TT : b * T + (tt + 1) * TT],
                    att_psum[:],
                )

        # scatter [1, BC*T] -> [BC, T] across partitions (one DMA per row;
        # a free->partition rearrange would NOT move data across physical
        # partitions on HW, even though CoreSim's linear memory accepts it)
        for b in range(BC):
            nc.sync.dma_start(
                att_sb[b : b + 1, :], att_row[0:1, b * T : (b + 1) * T]
            )

        # softmax over free dim (T) for all BC rows at once
        negmax = stat_pool.tile([BC, 1], f32)
        nc.vector.reduce_max(
            negmax[:], att_sb[:], axis=mybir.AxisListType.X, negate=True
        )
        exp_sb = const.tile([BC, T], f32)
        sums = stat_pool.tile([BC, 1], f32)
        nc.scalar.activation(
            exp_sb[:], att_sb[:], AF.Exp, bias=negmax[:], accum_out=sums[:]
        )
        recip = stat_pool.tile([BC, 1], f32)
        nc.vector.reciprocal(recip[:], sums[:])
        nc.vector.tensor_scalar_mul(exp_sb[:], exp_sb[:], recip[:])
        nc.sync.dma_start(out_d[:], exp_sb[:])

    nc.compile()
    return nc


_CACHED_NC = None


def _run(hidden, encoder_outputs, W, b, v, trace=False, **kw):
    from concourse.bass_utils import run_bass_kernel_spmd

    global _CACHED_NC
    if _CACHED_NC is None:
        _CACHED_NC = build_program()
    nc = _CACHED_NC

    hidden = np.asarray(hidden, dtype=np.float32)
    encoder_outputs = np.asarray(encoder_outputs, dtype=np.float32)
    W = np.asarray(W, dtype=np.float32)
    b = np.asarray(b, dtype=np.float32)
    v = np.asarray(v, dtype=np.float32)

    W1 = W[:, :H]
    W2 = W[:, H:]
    s1b = hidden @ W1.T + b  # [B, K]
    w2t = np.ascontiguousarray(W2.T)  # [H, K]
    # [T, B, H] -> [B, H, T]
    encT = np.ascontiguousarray(encoder_outputs.transpose(1, 2, 0))

    in_maps = []
    for c in range(NCORES):
        bs = slice(c * BC, (c + 1) * BC)
        in_maps.append(
            {
                "encT": encT[bs],
                "w2t": w2t,
                "s1b": np.ascontiguousarray(s1b[bs]),
                "v": v,
            }
        )

    res = run_bass_kernel_spmd(
        nc, in_maps, core_ids=list(range(NCORES)), trace=trace, **kw
    )
    out = np.concatenate([res.results[c]["out"] for c in range(NCORES)], axis=0)
    return out.reshape(B, 1, T).astype(np.float32), res


def kernel(hidden, encoder_outputs, W, b, v):
    return _run(hidden, encoder_outputs, W, b, v)[0]


# revision 11
# speedup vs baseline: 1.0193x; 1.0193x over previous
"""Bahdanau-style attention kernel for Trainium2, SPMD over 8 NeuronCores.

Problem (all fp32):
  hidden [B=32, H=1024], encoder_outputs [T=2048, B, H],
  W [H, 2H] (W1 | W2), b [H] (zeros), v [H]
  e    = tanh(hidden @ W1^T + enc @ W2^T + b)        [B, T, K=H]
  att  = e @ v                                       [B, T]
  out  = softmax(att, axis=T)[:, None, :]            [B, 1, T]

Sharding: data-parallel over B (4 batches per core), W/b/v replicated.

Per-core device algorithm (k on PSUM partitions, t on free dim):
  for b, t_tile(512), k_chunk(128):
      psum_e[k,t] = sum_{h_chunk} W2T[h,k].T @ encT[b][h,t]  (fp32r matmuls)
      e = tanh(psum_e + (s1[b]+bias)[k])                     (ACT, per-part bias)
      att_psum[1,t] += v[k_chunk].T @ e                      (fp32r matmul)
  softmax over t on rows {0,32,64,96} of a [128, T] tile, DMA out.

s1 = hidden @ W1^T (+b) is 0.05% of the FLOPs and is precomputed on host.
Weights/bias/v are pre-arranged on host so every DMA line is contiguous, and
the weight DMA is split per k-chunk so the first matmul can issue ~2us in.
"""

import numpy as np

B, T, H = 32, 2048, 1024
K = H
NCORES = 8
BC = B // NCORES  # batches per core
P = 128
HO = H // P       # 8 h-chunks
KO = K // P       # 8 k-chunks
TT = 512          # t tile (one PSUM bank of fp32)
NT = T // TT      # 4 t tiles


def build_program():
    from contextlib import ExitStack

    import concourse.tile as tile
    from concourse import bacc, mybir

    f32 = mybir.dt.float32
    f32r = mybir.dt.float32r
    AF = mybir.ActivationFunctionType

    nc = bacc.Bacc("TRN2", target_bir_lowering=False, debug=False)

    encT_d = nc.dram_tensor("encT", [BC, H, T], f32, kind="ExternalInput").ap()
    # host pre-arranged: w2t4[hp, ko, ho, kc] = W2[ko*128+kc, ho*128+hp]
    w2t4_d = nc.dram_tensor("w2t4", [P, KO, HO, P], f32, kind="ExternalInput").ap()
    # s1bd[kp, b*KO+ko] = (hidden @ W1.T + b)[b, ko*128+kp]
    s1bd_d = nc.dram_tensor("s1bd", [P, BC * KO], f32, kind="ExternalInput").ap()
    # vd[kp, ko] = v[ko*128+kp]
    vd_d = nc.dram_tensor("vd", [P, KO], f32, kind="ExternalInput").ap()
    out_d = nc.dram_tensor("out", [BC, T], f32, kind="ExternalOutput").ap()

    with tile.TileContext(nc) as tc, ExitStack() as ctx:
        const = ctx.enter_context(tc.tile_pool(name="const", bufs=1))
        enc_pool = ctx.enter_context(tc.tile_pool(name="enc", bufs=3))
        e_pool = ctx.enter_context(tc.tile_pool(name="e", bufs=3))
        psum_pool = ctx.enter_context(tc.tile_pool(name="psum", bufs=2, space="PSUM"))
        att_psum_pool = ctx.enter_context(
            tc.tile_pool(name="attpsum", bufs=2, space="PSUM")
        )
        stat_pool = ctx.enter_context(tc.tile_pool(name="stat", bufs=1))

        def new_enc_tile(b, tt):
            # one tile per (b, tt), DMA'd as 8 per-ho slices so matmuls can
            # start before the whole 2MB tile has landed
            enc_sb = enc_pool.tile([P, HO, TT], f32r)
            src = encT_d[b][:, tt * TT : (tt + 1) * TT].rearrange(
                "(ho hp) t -> hp ho t", hp=P
            )
            for ho in range(HO):
                nc.sync.dma_start(
                    enc_sb[:, ho, :], src[:, ho, :].bitcast(f32r)
                )
            return enc_sb

        # first enc tile queued before the weights so the PE can start early
        enc_first = new_enc_tile(0, 0)

        # weights, split per-ko: matmul group ko waits only on its slice
        w2t_sb = const.tile([P, KO, HO, P], f32r)
        for ko in range(KO):
            nc.sync.dma_start(
                w2t_sb[:, ko], w2t4_d[:, ko].bitcast(f32r)
            )
        v_sb = const.tile([P, KO], f32r)
        nc.sync.dma_start(v_sb[:], vd_d.bitcast(f32r))
        s1b_sb = const.tile([P, BC * KO], f32)
        nc.sync.dma_start(s1b_sb[:], s1bd_d)

        # energies on partitions {0,32,64,96} of one [128, T] tile
        att4 = const.tile([P, T], f32)
        nc.vector.memset(att4[:], 0.0)

        for b in range(BC):
            for tt in range(NT):
                enc_sb = enc_first if (b, tt) == (0, 0) else new_enc_tile(b, tt)
                att_psum = att_psum_pool.tile([1, TT], f32)
                for ko in range(KO):
                    psum_e = psum_pool.tile([P, TT], f32)
                    for ho in range(HO):
                        nc.tensor.matmul(
                            psum_e[:],
                            w2t_sb[:, ko, ho, :],
                            enc_sb[:, ho, :],
                            start=(ho == 0),
                            stop=(ho == HO - 1),
                        )
                    e_sb = e_pool.tile([P, TT], f32r)
                    nc.scalar.activation(
                        e_sb[:],
                        psum_e[:],
                        AF.Tanh,
                        bias=s1b_sb[:, b * KO + ko : b * KO + ko + 1],
                    )
                    nc.tensor.matmul(
                        att_psum[:],
                        v_sb[:, ko : ko + 1],
                        e_sb[:],
                        start=(ko == 0),
                        stop=(ko == KO - 1),
                    )
                nc.vector.tensor_copy(
                    att4[32 * b : 32 * b + 1, tt * TT : (tt + 1) * TT],
                    att_psum[:],
                )

        # softmax along T on all 128 partitions at once (only rows 0/32/64/96
        # carry data; the rest are zeros and harmless)
        negmax = stat_pool.tile([P, 1], f32)
        nc.vector.reduce_max(
            negmax[:], att4[:], axis=mybir.AxisListType.X, negate=True
        )
        exp_sb = const.tile([P, T], f32)
        sums = stat_pool.tile([P, 1], f32)
        nc.scalar.activation(
            exp_sb[:], att4[:], AF.Exp, bias=negmax[:], accum_out=sums[:]
        )
        recip = stat_pool.tile([P, 1], f32)
        nc.vector.reciprocal(recip[:], sums[:])
        nc.vector.tensor_scalar_mul(exp_sb[:], exp_sb[:], recip[:])
        for b in range(BC):
            nc.sync.dma_start(out_d[b], exp_sb[32 * b : 32 * b + 1, :])

    nc.compile()
    return nc


_CACHED_NC = None


def _run(hidden, encoder_outputs, W, b, v, trace=False, **kw):
    from concourse.bass_utils import run_bass_kernel_spmd

    global _CACHED_NC
    if _CACHED_NC is None:
        _CACHED_NC = build_program()
    nc = _CACHED_NC

    hidden = np.asarray(hidden, dtype=np.float32)
    encoder_outputs = np.asarray(encoder_outputs, dtype=np.float32)
    W = np.asarray(W, dtype=np.float32)
    b = np.asarray(b, dtype=np.float32)
    v = np.asarray(v, dtype=np.float32)

    W1 = W[:, :H]
    W2 = W[:, H:]
    s1b = hidden @ W1.T + b  # [B, K]
    # w2t4[hp, ko, ho, kc] = W2[ko*128+kc, ho*128+hp]
    w2t4 = np.ascontiguousarray(
        W2.reshape(KO, P, HO, P).transpose(3, 0, 2, 1)
    )
    # s1bd[kp, b*KO+ko] = s1b[b, ko*128+kp]  (per core's b slice)
    vd = np.ascontiguousarray(v.reshape(KO, P).T)  # [128, KO]
    # [T, B, H] -> [B, H, T]
    encT = np.ascontiguousarray(encoder_outputs.transpose(1, 2, 0))

    in_maps = []
    for c in range(NCORES):
        bs = slice(c * BC, (c + 1) * BC)
        s1bd = np.ascontiguousarray(
            s1b[bs].reshape(BC, KO, P).transpose(2, 0, 1).reshape(P, BC * KO)
        )
        in_maps.append(
            {
                "encT": encT[bs],
                "w2t4": w2t4,
                "s1bd": s1bd,
                "vd": vd,
            }
        )

    res = run_bass_kernel_spmd(
        nc, in_maps, core_ids=list(range(NCORES)), trace=trace, **kw
    )
    out = np.concatenate([res.results[c]["out"] for c in range(NCORES)], axis=0)
    return out.reshape(B, 1, T).astype(np.float32), res


def kernel(hidden, encoder_outputs, W, b, v):
    return _run(hidden, encoder_outputs, W, b, v)[0]
